# revision 8
# baseline (speedup 1.0000x reference)
"""Block 8x8 2D-IDCT kernel for Trainium2 (Bass/Tile), 8-core data-parallel.

Full input x_dct (4,64,64,64,8,8) f32 is sharded along flattened (N,C) into
8 shards of 32 images; each core independently computes the 2D IDCT of its
32 images and writes (32,512,512); results are concatenated on host.

Per-core pipeline, one tile = 2 images; each partition p = (img, bh) holds
one full block-row (64 blocks x 64 coeffs = 4096 f32):
  SWDGE cast-load HBM f32 -> SBUF bf16 (2 MiB read, 1 MiB written)
  -> PE transpose (bf16, 1 cyc/row) 16x [128,128] per half-tile into a
     bf16 PSUM strip; partitions become the 128 coeffs of a block-pair
  -> DVE copy PSUM->SBUF (bf16, 2x mode)
  -> bf16 matmul per sub-tile: stationary = transposed data, moving = g2p
     (column-permuted kron(M,M) block-diag) -> out[pair, (d,i,g,j)] f32 PSUM
  -> ACT copy PSUM->SBUF reordered so free dim = (i, w)
  -> single 2 MiB DMA store of full (im,u) x (i,w) tile

All PE work in bf16 (4x faster than f32); f32 is only touched by DMA + ACT.
"""

import math
from contextlib import ExitStack

import numpy as np

import concourse.bass as bass
import concourse.mybir as mybir
import concourse.tile as tile
from concourse import bacc, masks
from concourse.bass_utils import run_bass_kernel_spmd

F32 = mybir.dt.float32
BF16 = mybir.dt.bfloat16

N_CORES = 8
IMGS = 32           # images per core
TILES = IMGS // 2   # 2 images per tile
P = 128
BLOCK = 8


def _make_idct_matrix(nb: int) -> np.ndarray:
    m = np.zeros((nb, nb), dtype=np.float64)
    for n in range(nb):
        for k in range(nb):
            alpha = math.sqrt(1.0 / nb) if k == 0 else math.sqrt(2.0 / nb)
            m[n, k] = alpha * math.cos(math.pi * (2 * n + 1) * k / (2 * nb))
    return m.astype(np.float32)


def _g2p_matrix(idct_mat: np.ndarray) -> np.ndarray:
    """g2p[g*64+k, i*16+g'*8+j] = (g==g') * G[i*8+j, k], G = kron(M, M).

    Row index = coefficient (block-of-pair g, coeff k); column order (i, g', j)
    is chosen so a matmul writing 128 contiguous PSUM columns lands pixels in
    (i, g, j) order, making the later ACT reorder a clean 3D access pattern.
    """
    m = np.asarray(idct_mat, dtype=np.float32)
    g = np.kron(m, m)  # g[(n,l),(k,m)] = M[n,k] * M[l,m]
    gt = g.T.reshape(64, BLOCK, BLOCK)  # [k, i, j]
    g2p = np.zeros((P, P), dtype=np.float32)
    for gg in range(2):
        g2p[gg * 64 : (gg + 1) * 64].reshape(64, 8, 2, 8)[:, :, gg, :] = gt
    return g2p


def _build_nc(tiles: int = TILES) -> bass.Bass:
    nc = bacc.Bacc("TRN2", target_bir_lowering=False, debug=False)

    x = nc.dram_tensor("x", [tiles, P, 4096], F32, kind="ExternalInput")
    g2p = nc.dram_tensor("g2p", [P, P], F32, kind="ExternalInput")
    out = nc.dram_tensor("out", [2 * tiles, 512, 512], F32, kind="ExternalOutput")
    # partition p = (im, u); per-partition free = (i, w) is 16 KiB contiguous
    # in DRAM at offset ((t*2+im)*512 + u*8)*512 floats -> one DMA per tile.
    outv = out[:].rearrange(
        "(t im) (u i) w -> t (im u) (i w)", t=tiles, im=2, u=64, i=8
    )

    with tile.TileContext(nc) as tc, ExitStack() as ctx:
        consts = ctx.enter_context(tc.tile_pool(name="consts", bufs=1))
        lpool = ctx.enter_context(tc.tile_pool(name="load", bufs=3))
        cpool = ctx.enter_context(tc.tile_pool(name="conv", bufs=3))
        s1pool = ctx.enter_context(tc.tile_pool(name="s1", bufs=4))
        s3pool = ctx.enter_context(tc.tile_pool(name="s3", bufs=3))
        tpp = ctx.enter_context(
            tc.tile_pool(name="tp", bufs=2, space=bass.MemorySpace.PSUM)
        )
        pop = ctx.enter_context(
            tc.tile_pool(name="po", bufs=4, space=bass.MemorySpace.PSUM)
        )

        identb = consts.tile([P, P], BF16)
        masks.make_identity(nc, identb[:])
        g2pb = consts.tile([P, P], BF16)
        nc.gpsimd.dma_start(g2pb[:], g2p[:])  # cast f32 -> bf16 in DMA

        for t in range(tiles):
            # HWDGE f32 load on the ACT ring (avoids SWDGE descriptor-ring
            # contention that intermittently slows SDMA engines 7/15);
            # GPSIMD (otherwise idle) does the f32 -> bf16 narrowing.
            L = lpool.tile([P, 4096], F32)
            nc.scalar.dma_start(L[:], x[:][t])
            Lb = cpool.tile([P, 4096], BF16)
            nc.gpsimd.tensor_copy(Lb[:], L[:])
            S3 = s3pool.tile([P, 4096], F32)
            # S3 free layout: i*512 + hq*64 + d*16 + (g,j)  (= i*512 + w)
            s3v = S3[:].rearrange(
                "p (i hq d gj) -> p hq i d gj", i=8, hq=8, d=4, gj=16
            )
            for h in range(2):
                Tp = tpp.tile([P, 2048], BF16)
                for s16 in range(16):
                    nc.tensor.transpose(
                        Tp[:, s16 * P : (s16 + 1) * P],
                        Lb[:, h * 2048 + s16 * P : h * 2048 + (s16 + 1) * P],
                        identb[:],
                    )
                S1 = s1pool.tile([P, 2048], BF16)
                nc.vector.tensor_copy(S1[:], Tp[:])
                for q in range(4):
                    O = pop.tile([P, 512], F32)
                    for d in range(4):
                        sq = q * 4 + d
                        nc.tensor.matmul(
                            O[:, d * P : (d + 1) * P],
                            S1[:, sq * P : (sq + 1) * P],
                            g2pb[:],
                            start=True,
                            stop=True,
                        )
                    ov = O[:].rearrange("p (d i gj) -> p i d gj", d=4, i=8, gj=16)
                    nc.scalar.copy(s3v[:, h * 4 + q], ov)
            nc.sync.dma_start(outv[t], S3[:])

    nc.finalize()
    return nc


def _run(x_dct, idct_mat, H, W, trace: bool = False, tmpdir: str | None = None):
    x = np.ascontiguousarray(np.asarray(x_dct, dtype=np.float32))
    assert x.shape == (4, 64, 64, 64, BLOCK, BLOCK), x.shape
    H = int(H)
    W = int(W)
    assert H == 512 and W == 512, (H, W)

    g2p = _g2p_matrix(idct_mat)
    xs = x.reshape(N_CORES, TILES, P, 4096)

    nc = _build_nc(TILES)
    in_maps = [{"x": xs[c], "g2p": g2p} for c in range(N_CORES)]
    res = run_bass_kernel_spmd(
        nc, in_maps, core_ids=list(range(N_CORES)), trace=trace, tmpdir=tmpdir
    )
    outs = [res.results[c]["out"] for c in range(N_CORES)]
    full = np.concatenate(outs, axis=0).reshape(4, 64, 512, 512)
    return full[:, :, :H, :W], res


def kernel(x_dct, idct_mat=None, H=512, W=512):
    if idct_mat is None:
        idct_mat = _make_idct_matrix(BLOCK)
    out, _ = _run(x_dct, idct_mat, H, W, trace=False)
    return out


# revision 10
# speedup vs baseline: 2.1482x; 2.1482x over previous
"""Block 8x8 2D-IDCT kernel for Trainium2 (Bass/Tile), 8-core data-parallel.

Full input x_dct (4,64,64,64,8,8) f32 is sharded along flattened (N,C) into
8 shards of 32 images; each core independently computes the 2D IDCT of its
32 images and writes (32,512,512); results are concatenated on host.

Per-core pipeline, one tile = 2 images; each partition p = (img, bh) holds
one full block-row (64 blocks x 64 coeffs = 4096 f32):
  SWDGE cast-load HBM f32 -> SBUF bf16 (2 MiB read, 1 MiB written)
  -> PE transpose (bf16, 1 cyc/row) 16x [128,128] per half-tile into a
     bf16 PSUM strip; partitions become the 128 coeffs of a block-pair
  -> DVE copy PSUM->SBUF (bf16, 2x mode)
  -> bf16 matmul per sub-tile: stationary = transposed data, moving = g2p
     (column-permuted kron(M,M) block-diag) -> out[pair, (d,i,g,j)] f32 PSUM
  -> ACT copy PSUM->SBUF reordered so free dim = (i, w)
  -> single 2 MiB DMA store of full (im,u) x (i,w) tile

All PE work in bf16 (4x faster than f32); f32 is only touched by DMA + ACT.
"""

import math
from contextlib import ExitStack

import numpy as np

import concourse.bass as bass
import concourse.mybir as mybir
import concourse.tile as tile
from concourse import bacc, masks
from concourse.bass_utils import run_bass_kernel_spmd

F32 = mybir.dt.float32
BF16 = mybir.dt.bfloat16

N_CORES = 8
IMGS = 32           # images per core
TILES = IMGS // 2   # 2 images per tile
P = 128
BLOCK = 8


def _make_idct_matrix(nb: int) -> np.ndarray:
    m = np.zeros((nb, nb), dtype=np.float64)
    for n in range(nb):
        for k in range(nb):
            alpha = math.sqrt(1.0 / nb) if k == 0 else math.sqrt(2.0 / nb)
            m[n, k] = alpha * math.cos(math.pi * (2 * n + 1) * k / (2 * nb))
    return m.astype(np.float32)


def _g2p_matrix(idct_mat: np.ndarray) -> np.ndarray:
    """g2p[g*64+k, i*16+g'*8+j] = (g==g') * G[i*8+j, k], G = kron(M, M).

    Row index = coefficient (block-of-pair g, coeff k); column order (i, g', j)
    is chosen so a matmul writing 128 contiguous PSUM columns lands pixels in
    (i, g, j) order, making the later ACT reorder a clean 3D access pattern.
    """
    m = np.asarray(idct_mat, dtype=np.float32)
    g = np.kron(m, m)  # g[(n,l),(k,m)] = M[n,k] * M[l,m]
    gt = g.T.reshape(64, BLOCK, BLOCK)  # [k, i, j]
    g2p = np.zeros((P, P), dtype=np.float32)
    for gg in range(2):
        g2p[gg * 64 : (gg + 1) * 64].reshape(64, 8, 2, 8)[:, :, gg, :] = gt
    return g2p


def _build_nc(tiles: int = TILES) -> bass.Bass:
    nc = bacc.Bacc("TRN2", target_bir_lowering=False, debug=False)

    x = nc.dram_tensor("x", [tiles, P, 4096], F32, kind="ExternalInput")
    g2p = nc.dram_tensor("g2p", [P, P], F32, kind="ExternalInput")
    out = nc.dram_tensor("out", [2 * tiles, 512, 512], F32, kind="ExternalOutput")
    # partition p = (im, u); per-partition free = (i, w) is 16 KiB contiguous
    # in DRAM at offset ((t*2+im)*512 + u*8)*512 floats -> one DMA per tile.
    outv = out[:].rearrange(
        "(t im) (u i) w -> t (im u) (i w)", t=tiles, im=2, u=64, i=8
    )

    with tile.TileContext(nc) as tc, ExitStack() as ctx:
        consts = ctx.enter_context(tc.tile_pool(name="consts", bufs=1))
        lpool = ctx.enter_context(tc.tile_pool(name="load", bufs=3))
        s1pool = ctx.enter_context(tc.tile_pool(name="s1", bufs=4))
        # deep store-side buffering: bank finished output tiles so the store
        # stream keeps HBM saturated through the load->store transition tail
        s3pool = ctx.enter_context(tc.tile_pool(name="s3", bufs=6))
        tpp = ctx.enter_context(
            tc.tile_pool(name="tp", bufs=2, space=bass.MemorySpace.PSUM)
        )
        pop = ctx.enter_context(
            tc.tile_pool(name="po", bufs=4, space=bass.MemorySpace.PSUM)
        )

        identb = consts.tile([P, P], BF16)
        masks.make_identity(nc, identb[:])
        g2pb = consts.tile([P, P], BF16)
        nc.gpsimd.dma_start(g2pb[:], g2p[:])  # cast f32 -> bf16 in DMA

        for t in range(tiles):
            Lb = lpool.tile([P, 4096], BF16)
            nc.gpsimd.dma_start(Lb[:], x[:][t])  # cast f32 -> bf16 in DMA
            S3 = s3pool.tile([P, 4096], F32)
            # S3 free layout: i*512 + hq*64 + d*16 + (g,j)  (= i*512 + w)
            s3v = S3[:].rearrange(
                "p (i hq d gj) -> p hq i d gj", i=8, hq=8, d=4, gj=16
            )
            for h in range(2):
                Tp = tpp.tile([P, 2048], BF16)
                for s16 in range(16):
                    nc.tensor.transpose(
                        Tp[:, s16 * P : (s16 + 1) * P],
                        Lb[:, h * 2048 + s16 * P : h * 2048 + (s16 + 1) * P],
                        identb[:],
                    )
                S1 = s1pool.tile([P, 2048], BF16)
                nc.vector.tensor_copy(S1[:], Tp[:])
                for q in range(4):
                    O = pop.tile([P, 512], F32)
                    for d in range(4):
                        sq = q * 4 + d
                        nc.tensor.matmul(
                            O[:, d * P : (d + 1) * P],
                            S1[:, sq * P : (sq + 1) * P],
                            g2pb[:],
                            start=True,
                            stop=True,
                        )
                    ov = O[:].rearrange("p (d i gj) -> p i d gj", d=4, i=8, gj=16)
                    nc.scalar.copy(s3v[:, h * 4 + q], ov)
            nc.sync.dma_start(outv[t], S3[:])

    nc.finalize()
    return nc


def _run(x_dct, idct_mat, H, W, trace: bool = False, tmpdir: str | None = None):
    x = np.ascontiguousarray(np.asarray(x_dct, dtype=np.float32))
    assert x.shape == (4, 64, 64, 64, BLOCK, BLOCK), x.shape
    H = int(H)
    W = int(W)
    assert H == 512 and W == 512, (H, W)

    g2p = _g2p_matrix(idct_mat)
    xs = x.reshape(N_CORES, TILES, P, 4096)

    nc = _build_nc(TILES)
    in_maps = [{"x": xs[c], "g2p": g2p} for c in range(N_CORES)]
    res = run_bass_kernel_spmd(
        nc, in_maps, core_ids=list(range(N_CORES)), trace=trace, tmpdir=tmpdir
    )
    outs = [res.results[c]["out"] for c in range(N_CORES)]
    full = np.concatenate(outs, axis=0).reshape(4, 64, 512, 512)
    return full[:, :, :H, :W], res


def kernel(x_dct, idct_mat=None, H=512, W=512):
    if idct_mat is None:
        idct_mat = _make_idct_matrix(BLOCK)
    out, _ = _run(x_dct, idct_mat, H, W, trace=False)
    return out
